# revision 1
# baseline (speedup 1.0000x reference)
"""SimCLR NT-Xent contrastive loss on 8 Trainium2 cores.

Reference math (B=2048, D=256, T=0.5):
    zn = l2norm_rows(concat(z_i, z_j))          # [4096, 256]
    sim = zn @ zn.T / T                         # [4096, 4096]
    loss = mean_g [ log(sum_j exp(sim[g,j]) - exp(sim[g,g])) - sim[g, (g+B)%N] ]

Sharding: the 4096 sim rows are split across 8 cores (512 rows each).  Each
core receives the FULL z, rolled so that "its" rows sit at positions 0..511
and pre-transposed to [D, N] on the host (pure layout prep — no arithmetic).
With the roll, every core runs the identical static program:
  rows   = columns 0:512   of znT
  pos    = columns 2048:2560 of znT  (the (g+B)%N positive pairs)
Each core emits one fp32 partial: sum_g [log(rowsum_g - e^2) - 2*posdot_g];
the host sums the 8 partials and divides by 4096.  (sim[g,g] == 1/T exactly
for l2-normalized rows, so exp(diag) == e^2 up to fp32 noise ~1e-7.)

Per-core dataflow (all engines in play), pipelined per 512-column chunk in
four pair-groups so normalization streams right behind the input DMA:
  DMA    zt [256, 4096] fp32, column-chunked, interleaved [128, 2, 512]
  DVE    sq = zt*zt (bf16)
  PE     sumsq via M=32 all-ones matmul -> 32 replicated rows per chunk at
         partition 32r of a shared PSUM bank (K=256 over both d-halves)
  ACT    inv = exp(-0.5*ln(ss)) straight off PSUM (rsqrt via the single
         ln+exp table set; ACT Rsqrt is banned for accuracy)
  PE     replicate row 32r to all 128 partitions via a K=1 ones matmul
  DVE    znT = zt * invrep  (bf16)   -> normalized transposed z
  PE     sim block matmuls: lhsT = znT cols [rc*128..], rhs = znT col chunks,
         K=256 in 2 passes, PSUM [128, 1024] aligned with the pair-groups
  ACT    exp(2*sim) with accum_out -> per-row partial rowsums
  DVE    posdot via scalar_tensor_tensor(scale=-2) with accum_out
  ACT    log(rowsum - e^2)
  DVE+GPSIMD  reduce the [128, 6] tail -> [1, 1] partial, DMA out
"""

import numpy as np

import concourse.bacc as bacc
import concourse.bass as bass
import concourse.bass_isa as bass_isa
import concourse.tile as tile
from concourse import mybir

F32 = mybir.dt.float32
BF16 = mybir.dt.bfloat16
AF = mybir.ActivationFunctionType
ALU = mybir.AluOpType
AXIS = mybir.AxisListType

B = 2048
D = 256
N = 2 * B            # 4096 total rows
NCORES = 8
RPC = N // NCORES    # 512 rows per core
CC = 8               # column chunks of 512
CW = N // CC         # 512 chunk width
E2 = float(np.exp(np.float32(2.0)))   # exp(sim[g,g]) = exp(1/T)


class _Bacc(bacc.Bacc):
    """Bacc that pins the activation-table pass to the one set containing
    both Ln and Exp — the default fixpoint picks per-function sets and
    thrashes 5 table loads (~6.4us of ACT) into the schedule."""

    def insert_act_table_loads(self):
        from concourse.hw_specs import get_activation_tables
        import bass_rust as _bass_rust

        has_activation = any(
            isinstance(i, mybir.InstActivation)
            for b in self.main_func.blocks
            for i in b.instructions
        )
        if not has_activation:
            return
        # Keep the full list (act_func_set_id is the index into
        # act_info.json's act_func_sets!) but make the combined set the only
        # candidate for Ln/Exp so the pass can't alternate between
        # single-function sets.
        keep = {
            mybir.ActivationFunctionType.Ln,
            mybir.ActivationFunctionType.Exp,
        }
        tables = [
            (k, v if k == "natural_log_exp_and_others" else v - keep)
            for k, v in get_activation_tables(self.m.arch).items()
        ]
        _bass_rust.insert_act_table_loads(self, tables)


def build_nc():
    nc = _Bacc("TRN2", target_bir_lowering=False, debug=False)
    zt = nc.dram_tensor("zt", [D, N], F32, kind="ExternalInput").ap()
    out = nc.dram_tensor("out", [1, 1], F32, kind="ExternalOutput").ap()
    with tile.TileContext(nc) as tc:
        build_tile_program(tc, out, zt)
    nc.compile()
    return nc


def build_tile_program(tc: tile.TileContext, out: bass.AP, zt: bass.AP):
    nc = tc.nc
    # zt[d, n] viewed as [p, h, n] with d = h*128 + p
    zt_v = zt.rearrange("(h p) n -> p h n", h=2)

    with (
        tc.tile_pool(name="consts", bufs=1) as consts,
        tc.tile_pool(name="ztp", bufs=8) as ztp,
        tc.tile_pool(name="sqp", bufs=4) as sqp,
        tc.tile_pool(name="smalls", bufs=1) as smalls,
        tc.tile_pool(name="zntp", bufs=1) as zntp,
        tc.tile_pool(name="scrp", bufs=2) as scrp,
        tc.tile_pool(name="sspsp", bufs=1, space="PSUM") as sspsp,
        tc.tile_pool(name="invp", bufs=1, space="PSUM") as invp,
        tc.tile_pool(name="simp", bufs=3, space="PSUM") as simp,
    ):
        ones_sq = consts.tile([128, 128], F32, tag="ones_sq")
        nc.vector.memset(ones_sq, 1.0)
        ones_col32 = consts.tile([128, 32], BF16, tag="ones_col32")
        nc.vector.memset(ones_col32, 1.0)
        zero_col = consts.tile([128, 1], F32, tag="zero_col")
        nc.vector.memset(zero_col, 0.0)
        neg_e2 = consts.tile([128, 1], F32, tag="neg_e2")
        nc.vector.memset(neg_e2, -E2)

        # persistent small tiles
        znt = zntp.tile([128, 2, N], BF16)
        acc16 = smalls.tile([128, 16], F32, tag="acc16")
        tail6 = smalls.tile([128, 6], F32, tag="tail6")
        rowsum4 = smalls.tile([128, 4], F32, tag="rowsum4")
        tail1 = smalls.tile([128, 1], F32, tag="tail1")
        result = smalls.tile([128, 1], F32, tag="result")

        # ---- normalization pipeline.  Column chunks of 512 in four groups
        # of 2, so normalization streams right behind the input DMA and the
        # first sim matmuls (and ACT exps) start early.  Per chunk: sumsq
        # via an M=32 all-ones matmul (32 identical rows at partition 32r)
        # accumulating both d-halves in one PSUM bank.  Per group: rsqrt
        # straight off PSUM via exp(-0.5*ln); per chunk: K=1 matmul
        # replicates row 32r to all 128 partitions, then the normalize
        # multiplies produce bf16 znT.
        zt_tiles = {}
        sq_tiles = {}

        def load_and_sq(cc):
            ztc = ztp.tile([128, 2, CW], F32, tag="ztc", name=f"ztc{cc}")
            sqc = sqp.tile([128, 2, CW], BF16, tag="sqc", name=f"sqc{cc}")
            # stream each chunk in 256-column halves so square + sumsq
            # pipeline inside the DMA window instead of trailing it
            for q in range(2):
                qs = slice(q * 256, (q + 1) * 256)
                gqs = slice(cc * CW + q * 256, cc * CW + (q + 1) * 256)
                nc.sync.dma_start(out=ztc[:, :, qs], in_=zt_v[:, :, gqs])
                # split the square across DVE/GPSIMD so DVE (which also owns
                # the normalize multiplies) isn't the group-cadence governor
                nc.vector.tensor_mul(sqc[:, 0, qs], ztc[:, 0, qs], ztc[:, 0, qs])
                nc.gpsimd.tensor_mul(sqc[:, 1, qs], ztc[:, 1, qs], ztc[:, 1, qs])
            zt_tiles[cc] = ztc
            sq_tiles[cc] = sqc

        # software-pipelined emission: group g+1's load+sq are emitted (and
        # so FIFO-ordered) ahead of group g's dependent norm ops, avoiding
        # head-of-line stalls on the DVE queue
        load_and_sq(0)
        load_and_sq(1)
        for g in range(4):
            for r in range(2):
                nxt = 2 * (g + 1) + r
                if nxt < CC:
                    load_and_sq(nxt)
            ssps = sspsp.tile([128, CW], F32, tag="ssps")
            for r in range(2):
                cc = 2 * g + r
                for q in range(2):
                    qs = slice(q * 256, (q + 1) * 256)
                    for h in range(2):
                        nc.tensor.matmul(
                            ssps[32 * r : 32 * r + 32, qs],
                            ones_col32,
                            sq_tiles[cc][:, h, qs],
                            start=(h == 0),
                            stop=(h == 1),
                            tile_position=(0, 32 * r),
                        )

            # inv = exp(-0.5 * ln(sumsq)) = 1/sqrt(sumsq)  (ACT reads PSUM)
            lng = smalls.tile([64, CW], F32, tag="lng", bufs=2)
            invg = smalls.tile([64, CW], F32, tag="invg", bufs=2)
            nc.scalar.activation(lng, ssps[0:64, :], AF.Ln, bias=zero_col[0:64, :])
            nc.scalar.activation(
                invg, lng, AF.Exp, bias=zero_col[0:64, :], scale=-0.5
            )

            for r in range(2):
                cc = 2 * g + r
                cols = slice(cc * CW, (cc + 1) * CW)
                # replicate chunk cc's inv row (partition 32r) across all 128
                # partitions: K=1 outer product; lhsT/rhs share base 32r and
                # tile_position auto-derives to (32r, 0)
                invrep = invp.tile([128, CW], F32, tag="invrep")
                nc.tensor.matmul(
                    invrep,
                    ones_sq[32 * r : 32 * r + 1, :],
                    invg[32 * r : 32 * r + 1, :],
                    start=True,
                    stop=True,
                )
                # znT chunk = zt * invrep  (bf16), both d-halves
                for h in range(2):
                    nc.vector.tensor_mul(
                        znt[:, h, cols], zt_tiles[cc][:, h, :], invrep
                    )

        # ---- positive-pair dots: rows 0:512 vs rows 2048:2560
        # tail6[:, 4+h] = sum_i (-2 * znt[d, i]) * znt[d, 2048+i]
        for h in range(2):
            pd_scr = scrp.tile([128, RPC], BF16, tag="pd_scr")
            nc.vector.scalar_tensor_tensor(
                out=pd_scr,
                in0=znt[:, h, 0:RPC],
                scalar=-2.0,
                in1=znt[:, h, B : B + RPC],
                op0=ALU.mult,
                op1=ALU.mult,
                accum_out=tail6[:, 4 + h : 5 + h],
            )

        # ---- main matmul + exp + row-sum accumulation
        # col-group-major, with col groups (1024) aligned to the norm pair
        # groups so each group's matmuls start as soon as its own two chunks
        # are normalized.
        for cg in range(4):
            for rc in range(4):
                ps = simp.tile([128, 1024], F32, tag="ps")
                for h in range(2):
                    lhsT = znt[:, h, rc * 128 : (rc + 1) * 128]
                    for q in range(2):
                        cq = cg * 1024 + q * CW
                        nc.tensor.matmul(
                            ps[:, q * CW : (q + 1) * CW],
                            lhsT,
                            znt[:, h, cq : cq + CW],
                            start=(h == 0),
                            stop=(h == 1),
                        )
                scr = scrp.tile([128, 1024], BF16, tag="exp_scr")
                k = cg * 4 + rc
                nc.scalar.activation(
                    scr,
                    ps,
                    AF.Exp,
                    bias=zero_col,
                    scale=2.0,
                    accum_out=acc16[:, k : k + 1],
                )

        # ---- tail: rowsums, log(neg), total partial
        # acc16 col k = cg*4 + rc; rowsum4[:, rc] = sum_cg acc16[:, cg*4+rc]
        acc_v = acc16.rearrange("p (s r) -> p r s", s=4)
        nc.vector.tensor_reduce(
            out=rowsum4, in_=acc_v, axis=AXIS.X, op=ALU.add
        )
        # tail6[:, 0:4] = ln(rowsum - e^2)
        nc.scalar.activation(tail6[:, 0:4], rowsum4, AF.Ln, bias=neg_e2)
        # partial = sum over all partitions and columns of tail6
        nc.vector.tensor_reduce(out=tail1, in_=tail6, axis=AXIS.X, op=ALU.add)
        nc.gpsimd.partition_all_reduce(
            result, tail1, channels=128, reduce_op=bass_isa.ReduceOp.add
        )
        nc.sync.dma_start(out=out, in_=result[0:1, :])


_NC_CACHE = None


def _get_nc():
    global _NC_CACHE
    if _NC_CACHE is None:
        _NC_CACHE = build_nc()
    return _NC_CACHE


def make_in_maps(z_i: np.ndarray, z_j: np.ndarray):
    z = np.concatenate(
        [np.asarray(z_i, np.float32), np.asarray(z_j, np.float32)], axis=0
    )
    in_maps = []
    for c in range(NCORES):
        zr = np.roll(z, -RPC * c, axis=0)
        in_maps.append({"zt": np.ascontiguousarray(zr.T)})
    return in_maps


_EXEC_CACHE = None


def _get_exec():
    """Jitted 8-core SPMD executable, built once and reused across calls.

    Mirrors the multi-core tail of bass2jax.run_bass_via_pjrt but keeps the
    jitted function alive so repeated kernel() calls skip retrace/recompile.
    """
    global _EXEC_CACHE
    if _EXEC_CACHE is None:
        import jax
        from jax.experimental.shard_map import shard_map
        from jax.sharding import Mesh, PartitionSpec

        from concourse import bass2jax

        nc = _get_nc()
        bass2jax.install_neuronx_cc_hook()
        assert nc.dbg_addr is None
        part_name = (
            nc.partition_id_tensor.name if nc.partition_id_tensor else None
        )
        # input order: ExternalInputs, donated zeroed outputs, partition id
        in_names = ["zt", "out"] + ([part_name] if part_name else [])
        out_avals = (jax.core.ShapedArray((1, 1), np.float32),)

        def _body(*args):
            operands = list(args)
            if part_name is not None:
                operands.append(bass2jax.partition_id_tensor())
            outs = bass2jax._bass_exec_p.bind(
                *operands,
                out_avals=out_avals,
                in_names=tuple(in_names),
                out_names=("out",),
                lowering_input_output_aliases=(),
                sim_require_finite=True,
                sim_require_nnan=True,
                nc=nc,
            )
            return tuple(outs)

        devices = jax.devices()[:NCORES]
        mesh = Mesh(np.asarray(devices), ("core",))
        sharded = jax.jit(
            shard_map(
                _body,
                mesh=mesh,
                in_specs=(PartitionSpec("core"),) * 2,
                out_specs=(PartitionSpec("core"),),
                check_rep=False,
            ),
            donate_argnums=(1,),
            keep_unused=True,
        )
        _EXEC_CACHE = sharded
    return _EXEC_CACHE


def run_cores(in_maps):
    """Run the SPMD kernel; returns the 8 per-core [1,1] partials."""
    sharded = _get_exec()
    concat_zt = np.concatenate([m["zt"] for m in in_maps], axis=0)
    zeros = np.zeros((NCORES, 1), np.float32)
    (out,) = sharded(concat_zt, zeros)
    return np.asarray(out)  # [NCORES, 1]


def kernel(z_i: np.ndarray, z_j: np.ndarray) -> np.ndarray:
    partials = run_cores(make_in_maps(z_i, z_j))
    return np.float32(float(partials.sum()) / N)



# revision 10
# speedup vs baseline: 4.3300x; 4.3300x over previous
"""SimCLR NT-Xent contrastive loss on 8 Trainium2 cores, sharded inputs.

Reference math (B=2048, D=256, T=0.5):
    zn = l2norm_rows(concat(z_i, z_j))          # [4096, 256]
    sim = zn @ zn.T / T                         # [4096, 4096]
    loss = mean_g [ log(sum_j exp(sim[g,j]) - exp(sim[g,g])) - sim[g, (g+B)%N] ]

Sharding (data-parallel, the distributed-SimCLR pattern): core c receives
rows c*256..(c+1)*256 of BOTH z_i and z_j (512 of the 4096 global rows).
Host-to-device traffic is the 4 MB of raw shards — 8x less than replicating
z to every core — and there is no host-side prep at all (shard_map slices
the input arrays directly).

Because each core owns matching z_i/z_j slices, every positive pair
(g, g+B) is core-LOCAL: posdot_g never needs remote data.  The only
communication is one 256 KB-per-core AllGather of the locally normalized,
transposed, bf16 shards (split into z_i/z_j halves so the second gather
overlaps the first gather's matmuls), after which each core computes the
sim rows for its own 512 rows against all 4096 columns:

  DMA    zi, zj [256, 256] f32 each
  DVE    sumsq per row via scalar_tensor_tensor accum (free-axis)
  ACT    inv = exp(-0.5*ln(ss))  (rsqrt via the ln+exp table set; ACT
         Rsqrt is banned for accuracy)
  DVE    zn = z * inv (per-partition scalar broadcast) -> bf16
  DVE    posdot via scalar_tensor_tensor(scale=-4) with accum_out
  PE     transpose zn [rows, D] -> znT [D, rows] via identity matmuls
  DMA    bounce znT -> DRAM; GPSIMD AllGather (x2: zi half, zj half)
  DMA    gathered blocks -> SBUF as they land
  PE     sim block matmuls: lhsT = own znT m-chunks, rhs = gathered cols,
         K=256 in 2 passes, PSUM [128, 512]
  ACT    exp(2*sim) with accum_out -> per-own-row partial rowsums
  ACT    log(rowsum - e^2)   (sim[g,g] == 1/T exactly for l2-normalized
         rows, so exp(diag) == e^2 up to bf16 normalization noise)
  DVE+GPSIMD  reduce -> [1, 1] partial, DMA out

Each core emits one fp32 partial: sum_{own g} log(rowsum_g - e^2)
- 4*sum_{local pairs} posdot; the host sums the 8 partials and divides
by 4096.
"""

import numpy as np

import concourse.bacc as bacc
import concourse.bass as bass
import concourse.bass_isa as bass_isa
import concourse.tile as tile
from concourse import mybir
from concourse.masks import make_identity

F32 = mybir.dt.float32
BF16 = mybir.dt.bfloat16
AF = mybir.ActivationFunctionType
ALU = mybir.AluOpType
AXIS = mybir.AxisListType

B = 2048
D = 256
N = 2 * B            # 4096 total rows
NCORES = 8
LR = B // NCORES     # 256 local rows per input tensor
LN = 2 * LR          # 512 own rows per core
E2 = float(np.exp(np.float32(2.0)))   # exp(sim[g,g]) = exp(1/T)


class _Bacc(bacc.Bacc):
    """Bacc that pins the activation-table pass to the one set containing
    both Ln and Exp — the default fixpoint picks per-function sets and
    thrashes table loads into the schedule."""

    def insert_act_table_loads(self):
        from concourse.hw_specs import get_activation_tables
        import bass_rust as _bass_rust

        has_activation = any(
            isinstance(i, mybir.InstActivation)
            for b in self.main_func.blocks
            for i in b.instructions
        )
        if not has_activation:
            return
        keep = {
            mybir.ActivationFunctionType.Ln,
            mybir.ActivationFunctionType.Exp,
        }
        tables = [
            (k, v if k == "natural_log_exp_and_others" else v - keep)
            for k, v in get_activation_tables(self.m.arch).items()
        ]
        _bass_rust.insert_act_table_loads(self, tables)


def build_nc():
    nc = _Bacc(
        "TRN2", target_bir_lowering=False, debug=False, num_devices=NCORES
    )
    zi = nc.dram_tensor("zi", [LR, D], F32, kind="ExternalInput").ap()
    zj = nc.dram_tensor("zj", [LR, D], F32, kind="ExternalInput").ap()
    out = nc.dram_tensor("out", [1, 1], F32, kind="ExternalOutput").ap()
    with tile.TileContext(nc) as tc:
        build_tile_program(tc, out, zi, zj)
    nc.compile()
    return nc


def build_tile_program(tc: tile.TileContext, out: bass.AP, zi: bass.AP, zj: bass.AP):
    nc = tc.nc
    # [256, 256] viewed as [p, q, d] with row r = q*128 + p
    zi_v = zi.rearrange("(q p) d -> p q d", p=128)
    zj_v = zj.rearrange("(q p) d -> p q d", p=128)

    with (
        tc.tile_pool(name="consts", bufs=1) as consts,
        tc.tile_pool(name="inp", bufs=1) as inp,
        tc.tile_pool(name="scr", bufs=4) as scr,
        tc.tile_pool(name="smalls", bufs=1) as smalls,
        tc.tile_pool(name="znp", bufs=1) as znp,
        tc.tile_pool(name="sgp", bufs=1) as sgp,
        tc.tile_pool(name="escr", bufs=3) as escr,
        tc.tile_pool(name="tpsum", bufs=2, space="PSUM") as tpsum,
        tc.tile_pool(name="simp", bufs=3, space="PSUM") as simp,
        tc.tile_pool(name="dram", bufs=1, space="DRAM") as dram,
    ):
        identity = consts.tile([128, 128], BF16, tag="identity")
        make_identity(nc, identity)
        zero_col = consts.tile([128, 1], F32, tag="zero_col")
        nc.vector.memset(zero_col, 0.0)
        neg_e2 = consts.tile([128, 1], F32, tag="neg_e2")
        nc.vector.memset(neg_e2, -E2)

        zli = inp.tile([128, 2, D], F32, tag="zli")
        zlj = inp.tile([128, 2, D], F32, tag="zlj")
        ss4 = smalls.tile([128, 4], F32, tag="ss4")
        ln4 = smalls.tile([128, 4], F32, tag="ln4")
        inv4 = smalls.tile([128, 4], F32, tag="inv4")
        zni = znp.tile([128, 2, D], BF16, tag="zni")
        znj = znp.tile([128, 2, D], BF16, tag="znj")
        znTi = znp.tile([128, 2, LR], BF16, tag="znTi")
        znTj = znp.tile([128, 2, LR], BF16, tag="znTj")
        sgi = sgp.tile([128, 2, B], BF16, tag="sgi")
        sgj = sgp.tile([128, 2, B], BF16, tag="sgj")
        acc32 = smalls.tile([128, 32], F32, tag="acc32")
        rowsum4 = smalls.tile([128, 4], F32, tag="rowsum4")
        tail6 = smalls.tile([128, 6], F32, tag="tail6")
        tail1 = smalls.tile([128, 1], F32, tag="tail1")
        result = smalls.tile([128, 1], F32, tag="result")

        bounce_i = dram.tile([128, 2, LR], BF16, tag="bounce_i")
        bounce_j = dram.tile([128, 2, LR], BF16, tag="bounce_j")
        gth_i = dram.tile([NCORES, 128, 2, LR], BF16, tag="gth_i", addr_space="Shared")
        gth_j = dram.tile([NCORES, 128, 2, LR], BF16, tag="gth_j", addr_space="Shared")

        nc.sync.dma_start(out=zli, in_=zi_v)
        nc.sync.dma_start(out=zlj, in_=zj_v)

        # ---- normalize + posdot + transpose + gather, zi half then zj half
        for t, (zl, zn, znT, bounce, gth) in enumerate(
            (
                (zli, zni, znTi, bounce_i, gth_i),
                (zlj, znj, znTj, bounce_j, gth_j),
            )
        ):
            for q in range(2):
                sqs = scr.tile([128, D], BF16, tag="sq_scr", name=f"sqs{t}{q}")
                nc.vector.scalar_tensor_tensor(
                    out=sqs,
                    in0=zl[:, q, :],
                    scalar=1.0,
                    in1=zl[:, q, :],
                    op0=ALU.mult,
                    op1=ALU.mult,
                    accum_out=ss4[:, 2 * t + q : 2 * t + q + 1],
                )
            cs = slice(2 * t, 2 * t + 2)
            # inv = exp(-0.5 * ln(sumsq)) = 1/sqrt(sumsq)
            nc.scalar.activation(ln4[:, cs], ss4[:, cs], AF.Ln, bias=zero_col)
            nc.scalar.activation(
                inv4[:, cs], ln4[:, cs], AF.Exp, bias=zero_col, scale=-0.5
            )
            for q in range(2):
                nc.vector.tensor_scalar(
                    out=zn[:, q, :],
                    in0=zl[:, q, :],
                    scalar1=inv4[:, 2 * t + q : 2 * t + q + 1],
                    scalar2=None,
                    op0=ALU.mult,
                )
            if t == 1:
                # positive pairs are local: tail6[:, 4+q] = -4 * zni.znj
                for q in range(2):
                    ps_scr = scr.tile([128, D], BF16, tag="pos_scr", name=f"pos{q}")
                    nc.vector.scalar_tensor_tensor(
                        out=ps_scr,
                        in0=zni[:, q, :],
                        scalar=-4.0,
                        in1=znj[:, q, :],
                        op0=ALU.mult,
                        op1=ALU.mult,
                        accum_out=tail6[:, 4 + q : 5 + q],
                    )
            # PE transpose [rows, D] -> znT [D, rows] blocks
            for q in range(2):
                for h in range(2):
                    pt = tpsum.tile([128, 128], BF16, tag="pt", name=f"pt{t}{q}{h}")
                    nc.tensor.transpose(pt, zn[:, q, h * 128 : (h + 1) * 128], identity)
                    nc.vector.tensor_copy(
                        znT[:, h, q * 128 : (q + 1) * 128], pt
                    )
            nc.sync.dma_start(out=bounce, in_=znT)
            nc.gpsimd.collective_compute(
                "AllGather",
                ALU.bypass,
                replica_groups=[list(range(NCORES))],
                ins=[bounce.opt()],
                outs=[gth.opt()],
            )

        # gathered blocks -> SBUF, per source core so matmuls can stream
        for gth, sg in ((gth_i, sgi), (gth_j, sgj)):
            for k in range(NCORES):
                nc.sync.dma_start(
                    out=sg[:, :, k * LR : (k + 1) * LR], in_=gth[k]
                )

        # ---- sim matmuls + exp row-sum accumulation
        # out[m, n]: m = own row (4 chunks of 128), n = gathered column.
        for t, sg in enumerate((sgi, sgj)):
            for pair in range(4):
                for mc in range(4):
                    lhs_tile = znTi if mc < 2 else znTj
                    ps = simp.tile([128, 512], F32, tag="ps", name=f"ps{t}{pair}{mc}")
                    for h in range(2):
                        nc.tensor.matmul(
                            ps,
                            lhs_tile[:, h, (mc % 2) * 128 : (mc % 2 + 1) * 128],
                            sg[:, h, pair * 512 : (pair + 1) * 512],
                            start=(h == 0),
                            stop=(h == 1),
                        )
                    esc = escr.tile([128, 512], BF16, tag="esc", name=f"esc{t}{pair}{mc}")
                    col = (4 * t + pair) * 4 + mc
                    nc.scalar.activation(
                        esc,
                        ps,
                        AF.Exp,
                        bias=zero_col,
                        scale=2.0,
                        accum_out=acc32[:, col : col + 1],
                    )

        # ---- tail: rowsums, log(neg), total partial
        # acc32 col = s*4 + mc with s = (t, pair); rowsum4[:, mc] = sum_s
        acc_v = acc32.rearrange("p (s r) -> p r s", s=8)
        nc.vector.tensor_reduce(out=rowsum4, in_=acc_v, axis=AXIS.X, op=ALU.add)
        nc.scalar.activation(tail6[:, 0:4], rowsum4, AF.Ln, bias=neg_e2)
        nc.vector.tensor_reduce(out=tail1, in_=tail6, axis=AXIS.X, op=ALU.add)
        nc.gpsimd.partition_all_reduce(
            result, tail1, channels=128, reduce_op=bass_isa.ReduceOp.add
        )
        nc.sync.dma_start(out=out, in_=result[0:1, :])


_NC_CACHE = None


def _get_nc():
    global _NC_CACHE
    if _NC_CACHE is None:
        _NC_CACHE = build_nc()
    return _NC_CACHE


def make_in_maps(z_i: np.ndarray, z_j: np.ndarray):
    z_i = np.asarray(z_i, np.float32)
    z_j = np.asarray(z_j, np.float32)
    return [
        {
            "zi": np.ascontiguousarray(z_i[c * LR : (c + 1) * LR]),
            "zj": np.ascontiguousarray(z_j[c * LR : (c + 1) * LR]),
        }
        for c in range(NCORES)
    ]


_EXEC_CACHE = None


def _get_exec():
    """Jitted 8-core SPMD executable, built once and reused across calls."""
    global _EXEC_CACHE
    if _EXEC_CACHE is None:
        import jax
        from jax.experimental.shard_map import shard_map
        from jax.sharding import Mesh, PartitionSpec

        from concourse import bass2jax

        nc = _get_nc()
        bass2jax.install_neuronx_cc_hook()
        assert nc.dbg_addr is None
        part_name = (
            nc.partition_id_tensor.name if nc.partition_id_tensor else None
        )
        # input order: ExternalInputs, donated zeroed outputs, partition id
        in_names = ["zi", "zj", "out"] + ([part_name] if part_name else [])
        out_avals = (jax.core.ShapedArray((1, 1), np.float32),)

        def _body(*args):
            operands = list(args)
            if part_name is not None:
                operands.append(bass2jax.partition_id_tensor())
            outs = bass2jax._bass_exec_p.bind(
                *operands,
                out_avals=out_avals,
                in_names=tuple(in_names),
                out_names=("out",),
                lowering_input_output_aliases=(),
                sim_require_finite=True,
                sim_require_nnan=True,
                nc=nc,
            )
            return tuple(outs)

        devices = jax.devices()[:NCORES]
        mesh = Mesh(np.asarray(devices), ("core",))
        sharded = jax.jit(
            shard_map(
                _body,
                mesh=mesh,
                in_specs=(PartitionSpec("core"),) * 3,
                out_specs=(PartitionSpec("core"),),
                check_rep=False,
            ),
            donate_argnums=(2,),
            keep_unused=True,
        )
        _EXEC_CACHE = sharded
    return _EXEC_CACHE


def kernel(z_i: np.ndarray, z_j: np.ndarray) -> np.ndarray:
    sharded = _get_exec()
    zeros = np.zeros((NCORES, 1), np.float32)
    (out,) = sharded(
        np.asarray(z_i, np.float32), np.asarray(z_j, np.float32), zeros
    )
    return np.float32(float(np.asarray(out).sum()) / N)


# revision 23
# speedup vs baseline: 4.6065x; 1.0639x over previous
"""SimCLR NT-Xent contrastive loss on 8 Trainium2 cores, sharded inputs.

Reference math (B=2048, D=256, T=0.5):
    zn = l2norm_rows(concat(z_i, z_j))          # [4096, 256]
    sim = zn @ zn.T / T                         # [4096, 4096]
    loss = mean_g [ log(sum_j exp(sim[g,j]) - exp(sim[g,g])) - sim[g, (g+B)%N] ]

Sharding (data-parallel, the distributed-SimCLR pattern): core c receives
rows c*256..(c+1)*256 of BOTH z_i and z_j (512 of the 4096 global rows).
Host-to-device traffic is the 4 MB of raw shards — 8x less than replicating
z to every core — and there is no host-side prep at all (shard_map slices
the input arrays directly).

Because each core owns matching z_i/z_j slices, every positive pair
(g, g+B) is core-LOCAL: posdot_g never needs remote data.  The only
communication is one 256 KB-per-core AllGather of the locally normalized,
transposed, bf16 shards (split into z_i/z_j halves so the second gather
overlaps the first gather's matmuls), after which each core computes the
sim rows for its own 512 rows against all 4096 columns:

  DMA    zi, zj [256, 256] f32 each
  DVE    sumsq per row via scalar_tensor_tensor accum (free-axis)
  ACT    inv = exp(-0.5*ln(ss))  (rsqrt via the ln+exp table set; ACT
         Rsqrt is banned for accuracy)
  DVE    zn = z * inv (per-partition scalar broadcast) -> bf16
  DVE    posdot via scalar_tensor_tensor(scale=-4) with accum_out
  PE     transpose zn [rows, D] -> znT [D, rows] via identity matmuls
  DMA    bounce znT -> DRAM; GPSIMD AllGather (x2: zi half, zj half)
  DMA    gathered blocks -> SBUF as they land
  PE     sim block matmuls: lhsT = own znT m-chunks, rhs = gathered cols,
         K=256 in 2 passes, PSUM [128, 512]
  ACT    exp(2*sim) with accum_out -> per-own-row partial rowsums
  ACT    log(rowsum - e^2)   (sim[g,g] == 1/T exactly for l2-normalized
         rows, so exp(diag) == e^2 up to bf16 normalization noise)
  DVE+GPSIMD  reduce -> [1, 1] partial, DMA out

Each core emits one fp32 partial: sum_{own g} log(rowsum_g - e^2)
- 4*sum_{local pairs} posdot; the host sums the 8 partials and divides
by 4096.
"""

import numpy as np

import concourse.bacc as bacc
import concourse.bass as bass
import concourse.bass_isa as bass_isa
import concourse.tile as tile
from concourse import mybir
from concourse.masks import make_identity

F32 = mybir.dt.float32
BF16 = mybir.dt.bfloat16
FP8 = mybir.dt.float8e4
AF = mybir.ActivationFunctionType
ALU = mybir.AluOpType
AXIS = mybir.AxisListType

B = 2048
D = 256
N = 2 * B            # 4096 total rows
NCORES = 8
LR = B // NCORES     # 256 local rows per input tensor
LN = 2 * LR          # 512 own rows per core
E2 = float(np.exp(np.float32(2.0)))   # exp(sim[g,g]) = exp(1/T)


class _Bacc(bacc.Bacc):
    """Bacc that pins the activation-table pass to the one set containing
    both Ln and Exp — the default fixpoint picks per-function sets and
    thrashes table loads into the schedule."""

    def insert_act_table_loads(self):
        from concourse.hw_specs import get_activation_tables
        import bass_rust as _bass_rust

        has_activation = any(
            isinstance(i, mybir.InstActivation)
            for b in self.main_func.blocks
            for i in b.instructions
        )
        if not has_activation:
            return
        keep = {
            mybir.ActivationFunctionType.Ln,
            mybir.ActivationFunctionType.Exp,
        }
        tables = [
            (k, v if k == "natural_log_exp_and_others" else v - keep)
            for k, v in get_activation_tables(self.m.arch).items()
        ]
        _bass_rust.insert_act_table_loads(self, tables)


def build_nc():
    nc = _Bacc(
        "TRN2", target_bir_lowering=False, debug=False, num_devices=NCORES
    )
    zi = nc.dram_tensor("zi", [LR, D], BF16, kind="ExternalInput").ap()
    zj = nc.dram_tensor("zj", [LR, D], BF16, kind="ExternalInput").ap()
    out = nc.dram_tensor("out", [1, 1], F32, kind="ExternalOutput").ap()
    with tile.TileContext(nc) as tc:
        build_tile_program(tc, out, zi, zj)
    nc.compile()
    return nc


def build_tile_program(tc: tile.TileContext, out: bass.AP, zi: bass.AP, zj: bass.AP):
    nc = tc.nc
    # [256, 256] viewed as [p, q, d] with row r = q*128 + p
    zi_v = zi.rearrange("(q p) d -> p q d", p=128)
    zj_v = zj.rearrange("(q p) d -> p q d", p=128)

    with (
        tc.tile_pool(name="consts", bufs=1) as consts,
        tc.tile_pool(name="inp", bufs=1) as inp,
        tc.tile_pool(name="scr", bufs=4) as scr,
        tc.tile_pool(name="smalls", bufs=1) as smalls,
        tc.tile_pool(name="znp", bufs=1) as znp,
        tc.tile_pool(name="sgp", bufs=1) as sgp,
        tc.tile_pool(name="escr", bufs=2) as escr,
        tc.tile_pool(name="tpsum", bufs=2, space="PSUM") as tpsum,
        tc.tile_pool(name="simp", bufs=3, space="PSUM") as simp,
        tc.tile_pool(name="dram", bufs=1, space="DRAM") as dram,
    ):
        identity = consts.tile([128, 128], BF16, tag="identity")
        make_identity(nc, identity)
        zero_col = consts.tile([128, 1], F32, tag="zero_col")
        nc.vector.memset(zero_col, 0.0)
        neg_e2 = consts.tile([128, 1], F32, tag="neg_e2")
        nc.vector.memset(neg_e2, -E2)

        zli = inp.tile([128, 2, D], BF16, tag="zli")
        zlj = inp.tile([128, 2, D], BF16, tag="zlj")
        ss4 = smalls.tile([128, 4], F32, tag="ss4")
        ln4 = smalls.tile([128, 4], F32, tag="ln4")
        inv4 = smalls.tile([128, 4], F32, tag="inv4")
        zni = znp.tile([128, 2, D], BF16, tag="zni")
        znj = znp.tile([128, 2, D], BF16, tag="znj")
        znT = znp.tile([128, 2, LN], FP8, tag="znT")
        sg = sgp.tile([128, 2, N], FP8, tag="sg")
        acc16 = smalls.tile([128, 16], F32, tag="acc16")
        rowsum4 = smalls.tile([128, 4], F32, tag="rowsum4")
        tail6 = smalls.tile([128, 6], F32, tag="tail6")
        tail1 = smalls.tile([128, 1], F32, tag="tail1")
        result = smalls.tile([128, 1], F32, tag="result")

        bounce = dram.tile([128, 2, LN], FP8, tag="bounce")
        gth = dram.tile([NCORES, 128, 2, LN], FP8, tag="gth", addr_space="Shared")

        nc.sync.dma_start(out=zli, in_=zi_v)
        nc.sync.dma_start(out=zlj, in_=zj_v)

        # ---- normalize + posdot + transpose.  Emission order keeps the DVE
        # queue free of head-of-line stalls: both halves' sumsqs first, then
        # the normalize multiplies, then the psum->fp8 copies.
        halves = ((zli, zni, 0), (zlj, znj, LR))
        for t, (zl, zn, coff) in enumerate(halves):
            for q in range(2):
                sqs = scr.tile([128, D], BF16, tag="sq_scr", name=f"sqs{t}{q}")
                nc.vector.scalar_tensor_tensor(
                    out=sqs,
                    in0=zl[:, q, :],
                    scalar=1.0,
                    in1=zl[:, q, :],
                    op0=ALU.mult,
                    op1=ALU.mult,
                    accum_out=ss4[:, 2 * t + q : 2 * t + q + 1],
                )
        for t, (zl, zn, coff) in enumerate(halves):
            cs = slice(2 * t, 2 * t + 2)
            # inv = exp(-0.5 * ln(sumsq)) = 1/sqrt(sumsq)
            nc.scalar.activation(ln4[:, cs], ss4[:, cs], AF.Ln, bias=zero_col)
            nc.scalar.activation(
                inv4[:, cs], ln4[:, cs], AF.Exp, bias=zero_col, scale=-0.5
            )
        for t, (zl, zn, coff) in enumerate(halves):
            for q in range(2):
                nc.vector.tensor_scalar(
                    out=zn[:, q, :],
                    in0=zl[:, q, :],
                    scalar1=inv4[:, 2 * t + q : 2 * t + q + 1],
                    scalar2=None,
                    op0=ALU.mult,
                )
        # PE transpose [rows, D] -> znT [D, rows] blocks (fp8 cast on copy)
        for t, (zl, zn, coff) in enumerate(halves):
            for q in range(2):
                for h in range(2):
                    pt = tpsum.tile([128, 128], BF16, tag="pt", name=f"pt{t}{q}{h}")
                    nc.tensor.transpose(pt, zn[:, q, h * 128 : (h + 1) * 128], identity)
                    nc.vector.tensor_copy(
                        znT[:, h, coff + q * 128 : coff + (q + 1) * 128], pt
                    )
        # positive pairs are local: tail6[:, 4+q] = -4 * zni.znj
        for q in range(2):
            ps_scr = scr.tile([128, D], BF16, tag="pos_scr", name=f"pos{q}")
            nc.vector.scalar_tensor_tensor(
                out=ps_scr,
                in0=zni[:, q, :],
                scalar=-4.0,
                in1=znj[:, q, :],
                op0=ALU.mult,
                op1=ALU.mult,
                accum_out=tail6[:, 4 + q : 5 + q],
            )

        # ---- one fp8 AllGather of the whole local znT block
        nc.sync.dma_start(out=bounce, in_=znT)
        nc.gpsimd.collective_compute(
            "AllGather",
            ALU.bypass,
            replica_groups=[list(range(NCORES))],
            ins=[bounce.opt()],
            outs=[gth.opt()],
        )

        # gathered blocks -> SBUF, per source core so matmuls can stream
        for k in range(NCORES):
            nc.sync.dma_start(out=sg[:, :, k * LN : (k + 1) * LN], in_=gth[k])

        # ---- sim matmuls + exp row-sum accumulation
        # out[m, n]: m = own row (4 chunks of 128), n = gathered column.
        # Two 512-col blocks share one 2-bank psum so a single [128, 1024]
        # exp amortizes the ~370 ns fixed ACT cost per instruction.
        for grp in range(4):
            for mc in range(4):
                ps = simp.tile([128, 1024], F32, tag="ps", name=f"ps{grp}{mc}")
                for half in range(2):
                    nb = 2 * grp + half
                    for h in range(2):
                        nc.tensor.matmul(
                            ps[:, half * 512 : (half + 1) * 512],
                            znT[:, h, mc * 128 : (mc + 1) * 128],
                            sg[:, h, nb * 512 : (nb + 1) * 512],
                            start=(h == 0),
                            stop=(h == 1),
                        )
                esc = escr.tile([128, 1024], BF16, tag="esc", name=f"esc{grp}{mc}")
                col = grp * 4 + mc
                nc.scalar.activation(
                    esc,
                    ps,
                    AF.Exp,
                    bias=zero_col,
                    scale=2.0,
                    accum_out=acc16[:, col : col + 1],
                )

        # ---- tail: rowsums, log(neg), total partial
        # acc16 col = grp*4 + mc; rowsum4[:, mc] = sum_grp
        acc_v = acc16.rearrange("p (s r) -> p r s", s=4)
        nc.vector.tensor_reduce(out=rowsum4, in_=acc_v, axis=AXIS.X, op=ALU.add)
        nc.scalar.activation(tail6[:, 0:4], rowsum4, AF.Ln, bias=neg_e2)
        nc.vector.tensor_reduce(out=tail1, in_=tail6, axis=AXIS.X, op=ALU.add)
        nc.gpsimd.partition_all_reduce(
            result, tail1, channels=128, reduce_op=bass_isa.ReduceOp.add
        )
        nc.sync.dma_start(out=out, in_=result[0:1, :])


_NC_CACHE = None


def _get_nc():
    global _NC_CACHE
    if _NC_CACHE is None:
        _NC_CACHE = build_nc()
    return _NC_CACHE


def _to_bf16(x: np.ndarray) -> np.ndarray:
    import ml_dtypes

    return np.asarray(x, np.float32).astype(ml_dtypes.bfloat16)


def make_in_maps(z_i: np.ndarray, z_j: np.ndarray):
    z_i = _to_bf16(z_i)
    z_j = _to_bf16(z_j)
    return [
        {
            "zi": np.ascontiguousarray(z_i[c * LR : (c + 1) * LR]),
            "zj": np.ascontiguousarray(z_j[c * LR : (c + 1) * LR]),
        }
        for c in range(NCORES)
    ]


_EXEC_CACHE = None


def _get_exec():
    """Jitted 8-core SPMD executable, built once and reused across calls."""
    global _EXEC_CACHE
    if _EXEC_CACHE is None:
        import jax
        from jax.experimental.shard_map import shard_map
        from jax.sharding import Mesh, PartitionSpec

        from concourse import bass2jax

        nc = _get_nc()
        bass2jax.install_neuronx_cc_hook()
        assert nc.dbg_addr is None
        part_name = (
            nc.partition_id_tensor.name if nc.partition_id_tensor else None
        )
        # input order: ExternalInputs, donated zeroed outputs, partition id
        in_names = ["zi", "zj", "out"] + ([part_name] if part_name else [])
        out_avals = (jax.core.ShapedArray((1, 1), np.float32),)

        def _body(*args):
            operands = list(args)
            if part_name is not None:
                operands.append(bass2jax.partition_id_tensor())
            outs = bass2jax._bass_exec_p.bind(
                *operands,
                out_avals=out_avals,
                in_names=tuple(in_names),
                out_names=("out",),
                lowering_input_output_aliases=(),
                sim_require_finite=True,
                sim_require_nnan=True,
                nc=nc,
            )
            return tuple(outs)

        devices = jax.devices()[:NCORES]
        mesh = Mesh(np.asarray(devices), ("core",))
        sharded = jax.jit(
            shard_map(
                _body,
                mesh=mesh,
                in_specs=(PartitionSpec("core"),) * 3,
                out_specs=(PartitionSpec("core"),),
                check_rep=False,
            ),
            donate_argnums=(2,),
            keep_unused=True,
        )
        _EXEC_CACHE = sharded
    return _EXEC_CACHE


def kernel(z_i: np.ndarray, z_j: np.ndarray) -> np.ndarray:
    sharded = _get_exec()
    zeros = np.zeros((NCORES, 1), np.float32)
    (out,) = sharded(_to_bf16(z_i), _to_bf16(z_j), zeros)
    return np.float32(float(np.asarray(out).sum()) / N)
